# revision 27
# baseline (speedup 1.0000x reference)
"""DeepSeekMoE block on 8 Trainium2 NeuronCores.

Sharding: expert-parallel — core e owns expert e's FFN (up_w[e]/down_w[e]);
tokens are dispatched to expert cores by host-side top-2 gating (the gate
matmul is 0.03% of total FLOPs).  The shared expert is token-parallel:
core e also runs the shared FFN for tokens [e*256, (e+1)*256).

Routed capacity is CAP=512 (one PSUM bank of columns); the few token-pairs
beyond an expert's capacity are computed on the host in fp32 (~1% FLOPs).

Device kernel per core (SPMD), routed + shared INTERLEAVED per iteration so
the weight-stream DMA demand is flat (~200 GB/s) instead of alternating
145/305 GB/s phases:
  up:   for ic: hact_r[ic] = gelu(up_w[e][:,ic].T @ xT + up_b)   [128,512]
               hact_s[ic] = gelu(sw_up[:,ic].T  @ xsT + sb_up)   [128,256]
  down: for hb: eoT[hb] = 0.1*(dn_w tiles @ hact_r + dn_b)       [128,512]
               soT[hb] = 0.1*(sw_dn tiles @ hact_s + sb_dn)      [128,256]
Matmuls in bf16 (fp32 PSUM accumulate); outputs written as bf16.
Routed and shared weights are packed interleaved per chunk on the host so
each up/down iteration is a single DMA; each output half ships as soon as
its activation lands so the post-stream tail is minimal.
A dummy-matmul warmup chain over real weight bits raises the Tensor
engine's DVFS p-state during the initial input-DMA window (the PE runs at
half clock until it has executed several microseconds of real-toggle work,
and all-zero operands do not ramp it).

fp8 was evaluated and rejected: e4m3 double-row matmuls (2x throughput,
measured on HW) cost 3.9e-2 rel err for even one stage (budget 2e-2), and
hi/lo split compensation needs 3 products per chunk pair = 1.5x bf16 time.

Host: gating/top-k (fp64 scores, fp32 combine weights), overflow-pair FFN,
scatter-add of the expert contributions + shared path, row max-abs norm.
"""
import sys
sys.path.insert(0, '/opt/trn_rl_repo')
import numpy as np
from contextlib import ExitStack

H = 1024
I = 4096
E = 8
TOPK = 2
B, S = 2, 1024
T = B * S            # 2048 tokens
CAP = 512            # routed-token capacity per expert core (overflow -> host)
TS = T // E          # shared-expert tokens per core = 256
HC = H // 128        # 8 h-chunks
IC = I // 128        # 32 i-chunks
XCH = 2              # xT initial-DMA chunks
WARM = 20            # dummy matmuls to raise PE DVFS while input DMAs land

_COMPILED = {}


def _build_nc():
    from concourse import bacc, tile, mybir

    F32 = mybir.dt.float32
    CDT = mybir.dt.bfloat16
    GELU = mybir.ActivationFunctionType.Gelu
    IDENT = mybir.ActivationFunctionType.Identity

    nc = bacc.Bacc("TRN2", target_bir_lowering=False, debug=False, num_devices=E)

    UW = 2 * HC * 128    # merged routed+shared up-weight block per ic
    DW = 2 * IC * 128    # merged routed+shared down-weight block per hb
    xT_d = nc.dram_tensor("xT", [128, HC * CAP], CDT, kind="ExternalInput")
    xsT_d = nc.dram_tensor("xsT", [128, HC * TS], CDT, kind="ExternalInput")
    uw_d = nc.dram_tensor("uw", [128, IC * UW], CDT, kind="ExternalInput")
    dw_d = nc.dram_tensor("dw", [128, HC * DW], CDT, kind="ExternalInput")
    ub_d = nc.dram_tensor("ub", [128, 2 * IC], F32, kind="ExternalInput")
    db_d = nc.dram_tensor("db", [128, 2 * HC], F32, kind="ExternalInput")
    coT_d = nc.dram_tensor("coT", [HC, 128, CAP + TS], CDT, kind="ExternalOutput")

    with tile.TileContext(nc) as tc, ExitStack() as ctx:
        pool = ctx.enter_context(tc.tile_pool(name="sbuf", bufs=1))
        uwpool = ctx.enter_context(tc.tile_pool(name="uw", bufs=6))
        dwpool = ctx.enter_context(tc.tile_pool(name="dw", bufs=4))
        hpool_r = ctx.enter_context(tc.tile_pool(name="hact_r", bufs=IC))
        hpool_s = ctx.enter_context(tc.tile_pool(name="hact_s", bufs=IC))
        opool = ctx.enter_context(tc.tile_pool(name="outs", bufs=4))
        ups_r = ctx.enter_context(tc.tile_pool(name="upsr", bufs=2, space="PSUM"))
        ups_s = ctx.enter_context(tc.tile_pool(name="upss", bufs=2, space="PSUM"))
        dns_r = ctx.enter_context(tc.tile_pool(name="dnsr", bufs=2, space="PSUM"))
        dns_s = ctx.enter_context(tc.tile_pool(name="dnss", bufs=2, space="PSUM"))

        # PE warmup: the Tensor engine starts at a low DVFS p-state and only
        # approaches max clock after microseconds of real-toggle execution
        # (all-zero operands do not ramp it).  Burn the input-DMA wait window
        # on dummy matmuls over a small slice of real weight data (random
        # bits toggle the array) so the p-state governor is already ramping
        # when the real chains start.  The 96KB seed is the first DMA issued.
        warm_t = pool.tile([128, 384], CDT, tag="warm")
        nc.sync.dma_start(warm_t[:], uw_d.ap()[:, 0:384])
        ps_w = ups_s.tile([128, TS], F32, tag="upss")
        for i in range(WARM):
            nc.tensor.matmul(ps_w[:], warm_t[:, 256:384], warm_t[:, 0:256],
                             start=(i == 0), stop=(i == WARM - 1))

        # resident activations + biases; uw0 + xT first (the routed chain 0
        # is the first consumer).  The down-proj biases are issued at the
        # top of the down loop instead of here.
        xT_t = pool.tile([128, HC * CAP], CDT, tag="xT")
        uw0 = uwpool.tile([128, UW], CDT, tag="upw")
        # uw0 ships as two halves so the routed chain 0 only waits for its
        # own 0.25MB, not the merged 0.5MB block
        nc.sync.dma_start(uw0[:, 0:UW // 2], uw_d.ap()[:, 0:UW // 2])
        xw = HC * CAP // XCH
        for xc in range(XCH):
            nc.sync.dma_start(xT_t[:, xc * xw:(xc + 1) * xw],
                              xT_d.ap()[:, xc * xw:(xc + 1) * xw])
        nc.sync.dma_start(uw0[:, UW // 2:UW], uw_d.ap()[:, UW // 2:UW])
        xsT_t = pool.tile([128, HC * TS], CDT, tag="xsT")
        nc.sync.dma_start(xsT_t[:], xsT_d.ap()[:])
        ub_t = pool.tile([128, 2 * IC], F32, tag="ub")
        nc.sync.dma_start(ub_t[:], ub_d.ap()[:])

        # ---- up projection + gelu, routed & shared interleaved per ic ----
        hacts_r, hacts_s = [], []
        for ic in range(IC):
            if ic == 0:
                uw = uw0
            else:
                uw = uwpool.tile([128, UW], CDT, tag="upw")
                nc.sync.dma_start(uw[:], uw_d.ap()[:, ic * UW:(ic + 1) * UW])

            ps = ups_r.tile([128, CAP], F32, tag="upsr")
            for hc in range(HC):
                nc.tensor.matmul(
                    ps[:],
                    uw[:, hc * 128:(hc + 1) * 128],
                    xT_t[:, hc * CAP:(hc + 1) * CAP],
                    start=(hc == 0), stop=(hc == HC - 1),
                )
            ht = hpool_r.tile([128, CAP], CDT, tag="hact")
            nc.scalar.activation(ht[:], ps[:], GELU, bias=ub_t[:, ic:ic + 1])
            hacts_r.append(ht)

            ps = ups_s.tile([128, TS], F32, tag="upss")
            for hc in range(HC):
                nc.tensor.matmul(
                    ps[:],
                    uw[:, HC * 128 + hc * 128:HC * 128 + (hc + 1) * 128],
                    xsT_t[:, hc * TS:(hc + 1) * TS],
                    start=(hc == 0), stop=(hc == HC - 1),
                )
            ht = hpool_s.tile([128, TS], CDT, tag="shact")
            nc.scalar.activation(ht[:], ps[:], GELU,
                                 bias=ub_t[:, IC + ic:IC + ic + 1])
            hacts_s.append(ht)

        # ---- down projection, routed & shared interleaved per hb ----
        db_t = pool.tile([128, 2 * HC], F32, tag="db")
        nc.sync.dma_start(db_t[:], db_d.ap()[:])
        for hb in range(HC):
            dw = dwpool.tile([128, DW], CDT, tag="dnw")
            nc.sync.dma_start(dw[:, 0:DW // 2],
                              dw_d.ap()[:, hb * DW:hb * DW + DW // 2])
            nc.sync.dma_start(dw[:, DW // 2:DW],
                              dw_d.ap()[:, hb * DW + DW // 2:(hb + 1) * DW])

            ot = opool.tile([128, CAP + TS], CDT, tag="out")
            ps = dns_r.tile([128, CAP], F32, tag="dnsr")
            for ic in range(IC):
                nc.tensor.matmul(
                    ps[:],
                    dw[:, ic * 128:(ic + 1) * 128],
                    hacts_r[ic][:],
                    start=(ic == 0), stop=(ic == IC - 1),
                )
            nc.scalar.activation(ot[:, 0:CAP], ps[:], IDENT,
                                 bias=db_t[:, hb:hb + 1], scale=0.1)
            # ship the routed half as soon as its activation lands: the
            # final DMA after the last (shared) chain is then only 0.5KB/row
            nc.sync.dma_start(coT_d.ap()[hb, :, 0:CAP], ot[:, 0:CAP])

            ps = dns_s.tile([128, TS], F32, tag="dnss")
            for ic in range(IC):
                nc.tensor.matmul(
                    ps[:],
                    dw[:, IC * 128 + ic * 128:IC * 128 + (ic + 1) * 128],
                    hacts_s[ic][:],
                    start=(ic == 0), stop=(ic == IC - 1),
                )
            nc.scalar.activation(ot[:, CAP:CAP + TS], ps[:], IDENT,
                                 bias=db_t[:, HC + hb:HC + hb + 1], scale=0.1)
            nc.sync.dma_start(coT_d.ap()[hb, :, CAP:CAP + TS],
                              ot[:, CAP:CAP + TS])

    nc.compile()
    return nc


def _get_compiled():
    if "nc" not in _COMPILED:
        _COMPILED["nc"] = _build_nc()
    return _COMPILED["nc"]


def _np_cdt():
    import ml_dtypes
    return np.dtype(ml_dtypes.bfloat16)


def _pack_weight(w):
    """[K, N] -> [128, (N/128 chunks) x (K/128 subtiles) x 128] stream layout."""
    kdim, ndim = w.shape
    kc, nchunk = kdim // 128, ndim // 128
    return np.ascontiguousarray(
        w.reshape(kc, 128, nchunk, 128).transpose(1, 2, 0, 3)
    ).reshape(128, nchunk * kc * 128).astype(_np_cdt())


def _merge_blocks(a, b, nblk):
    """Interleave two [128, nblk*W] packs into [128, nblk*2W] per-block."""
    w = a.shape[1] // nblk
    return np.ascontiguousarray(
        np.stack([a.reshape(128, nblk, w), b.reshape(128, nblk, w)],
                 axis=2).reshape(128, nblk * 2 * w))


def _pack_tokens(xsel, cap):
    """[n, H] tokens -> [128, HC*cap] transposed h-chunked layout, zero pad."""
    n = xsel.shape[0]
    arr = np.zeros((128, HC, cap), np.float32)
    if n:
        arr[:, :, :n] = xsel.T.reshape(HC, 128, n).transpose(1, 0, 2)
    return np.ascontiguousarray(arr).reshape(128, HC * cap).astype(_np_cdt())


def _pack_bias(b, scale=1.0):
    """[N] -> [128, N/128] per-partition layout."""
    return np.ascontiguousarray(
        (np.asarray(b, np.float32) * scale).reshape(-1, 128).T.astype(np.float32))


def _gelu(u):
    try:
        from scipy.special import erf
    except ImportError:
        import math
        erf = np.vectorize(math.erf, otypes=[np.float64])
    return (0.5 * u * (1.0 + erf(u / np.sqrt(2.0)))).astype(u.dtype)


def kernel(x, gate_w, bias, up_w, up_b, down_w, down_b,
           sw_up, sb_up, sw_down, sb_down):
    from concourse.bass_utils import run_bass_kernel_spmd

    x = np.asarray(x, np.float32)
    xf = x.reshape(T, H)

    # ---- host gating (fp64 scores for a stable top-k, fp32 combine weights)
    z64 = xf.astype(np.float64) @ np.asarray(gate_w, np.float64) \
        + np.asarray(bias, np.float64)
    scores64 = 1.0 / (1.0 + np.exp(-z64))
    top_idx = np.argsort(-scores64, axis=-1, kind="stable")[:, :TOPK]
    tsc = scores64[np.arange(T)[:, None], top_idx].astype(np.float32)
    wts = tsc / (tsc.sum(-1, keepdims=True) + np.float32(1e-6))   # [T, 2]

    # ---- token dispatch: first CAP tokens per expert on device, rest host
    tok_lists, over_lists = [], []
    for e in range(E):
        tl = np.where((top_idx == e).any(-1))[0]
        tok_lists.append(tl[:CAP])
        over_lists.append(tl[CAP:])

    supw = _pack_weight(np.asarray(sw_up, np.float32))
    sdnw = _pack_weight(np.asarray(sw_down, np.float32))
    supb = _pack_bias(sb_up)
    sdnb = _pack_bias(sb_down, scale=0.1)

    in_maps = []
    for e in range(E):
        in_maps.append({
            "xT": _pack_tokens(xf[tok_lists[e]], CAP),
            "xsT": _pack_tokens(xf[e * TS:(e + 1) * TS], TS),
            "uw": _merge_blocks(_pack_weight(np.asarray(up_w[e], np.float32)),
                                supw, IC),
            "dw": _merge_blocks(_pack_weight(np.asarray(down_w[e], np.float32)),
                                sdnw, HC),
            "ub": np.concatenate([_pack_bias(up_b[e]), supb], axis=1),
            "db": np.concatenate([_pack_bias(down_b[e], scale=0.1), sdnb],
                                 axis=1),
        })

    nc = _get_compiled()
    res = run_bass_kernel_spmd(nc, in_maps, list(range(E)))

    # ---- host combine: scatter-add expert outputs, add shared, normalize
    out = np.zeros((T, H), np.float32)
    for e in range(E):
        coT = np.asarray(res.results[e]["coT"], np.float32)  # [HC,128,CAP+TS]
        soT = coT[:, :, CAP:]                                # [HC, 128, TS]
        out[e * TS:(e + 1) * TS] = soT.reshape(H, TS).T
    for e in range(E):
        tl = tok_lists[e]
        if len(tl):
            coT = np.asarray(res.results[e]["coT"], np.float32)
            eo = coT[:, :, 0:CAP].reshape(H, CAP)[:, :len(tl)].T    # [n, H]
            we = np.where(top_idx[tl, 0] == e,
                          wts[tl, 0], wts[tl, 1]).astype(np.float32)
            out[tl] += we[:, None] * eo
        ol = over_lists[e]
        if len(ol):  # overflow pairs: exact fp32 FFN on host
            u = xf[ol] @ np.asarray(up_w[e], np.float32) \
                + np.asarray(up_b[e], np.float32)
            eo = (_gelu(u) @ np.asarray(down_w[e], np.float32)
                  + np.asarray(down_b[e], np.float32)) * np.float32(0.1)
            we = np.where(top_idx[ol, 0] == e,
                          wts[ol, 0], wts[ol, 1]).astype(np.float32)
            out[ol] += we[:, None] * eo

    out /= (np.abs(out).max(-1, keepdims=True) + np.float32(1e-6))
    return out.reshape(B, S, H)
